# revision 45
# baseline (speedup 1.0000x reference)
"""Multi-head attention kernel for Trainium2, 8 NeuronCores, data-parallel over batch.

Problem: batch=16, pos=577, d_model=1024, n_heads=16, d_head=64, fp32.
Sharding: batch across 8 cores (2 batch items per core), no collectives.

v17 (~259.3us HW at the chip's normal 2.4GHz power state; v13 ~260-261us,
v9 ~262-265us, earlier baseline 307us). NOTE: under sustained benching the
chip drops into P0 (PE ~2.0GHz, N=512 MM spacing 259ns instead of 216ns)
and the same kernel measures ~304-309us; ~2-3min of idle restores 2.4GHz.
Always check MM spacing in the trace before comparing runs. Key mechanics,
all trace-verified:
  - PE streams moving columns at 2.4GHz (measured: N=512 MMs at 216ns
    start-to-start = 512/2.4+2.5, N=385 at 163ns). Issue-cycle floor for
    this kernel is ~450k cols ~ 187us; consecutive matmuls must target
    different PSUM banks (else +drain barrier) -> every accumulation chain
    alternates banks between consecutive MMs.
  - v17 adds over v13: each A-pair's two PSUM accumulation chains are
    STAGGERED (tile j0 ends its k-chain one slot early; its DVE
    evacuation overlaps the j1k7 closer MM + one attention block), so the
    two serial DVE evacuations stop sitting on the next pair's PSUM-reuse
    critical path. Pairs are 5 fill units (h0-h2: 4 MMs, h3: j0k6/j1k6/
    j0k7 + evac0, closer: j1k7 + evac1); filler budget 8/emit, 2 pops max
    per site, pair-start gate at 650ns (550 and 800 both measured slower).
  - v13 adds over v9: (1) ~30 zero-input warmup MMs at t=0 (the DMA
    descriptor-table boot means no input byte lands before ~9us; warmups
    get HAM to K=8/8 early), (2) x/w DMA chunks interleaved across both HW
    queues so chunk k of both tensors lands at ~(k+1)/8 of the transfer,
    (3) m0's Q/K-proj quarters emitted interleaved into phase V's b1 half
    (B(0,0) starts ~8us earlier; pair boundaries covered by V MMs),
    (4) granular phase-C fills (per-3-hp prefix advance) + reserve b0
    units so the b1-hp7 normalization latency is covered by PE work,
    (5) out-writes alternate sync/scalar, wo prefetch split across both.
  - Measured dead ends (do NOT revisit): per-hd exp splitting (+20us ACT
    busy, exps are the attention pacer -> net loss); A-evac on ACT via
    activation(Identity, bias=...) (delays exps in the strict ACT FIFO);
    gpsimd.partition_broadcast for the denominator broadcast (1.1us/instr
    ucode cost + cross-engine ping-pong, net +24us; also its src AND dst
    APs only honor base partition 0); warm filler MMs between V groups
    (+4us); outs on gpsimd SWDGE (slow tail drain).
  - The dd/rb DMA roundtrip sits ~30-50us behind the input stream on the
    sync queue mid-loop (queue FIFO); harmless for the m-loop (z-scale is
    only needed by phase C) but it is why phase C can stall ~5us at
    ~210us. Known remaining stalls: ~25us spread through the m-loop
    (psA evacuation WAR: next pair's first MM waits sem+DVE-add+sem
    ~714ns; interleaved attention covers only ~645ns), ~8us teardown tail.
  - Score matmuls for the two heads of a pair are emitted adjacently with
    row-disjoint tile_position (K=64 each) -> concurrent in the PE array,
    ~2x effective rate.
  - q-chunks (512, 65). Scores for one kt land in a fresh 2-bank psS pool
    tile ([kszx512 | kszx512] h0|h1); exp is ONE flat [ksz,1024] ACT gulp
    per tile (352-cyc fixed cost amortized; PSUM regions are strictly
    write-once-read-once per pool-tile generation, which the dep tracker
    handles robustly).
  - Augmented V = [V_h | 1]: AV matmul row 64 = softmax denominator for
    free. Denominator rows are copied out with plain tensor_copy (NOTE:
    reciprocal_approx_fast reads the wrong partition when src/dst base
    partitions differ - copy first, then recip in place), reciprocal'd
    batched, broadcast via DRAM roundtrip, z scaled on GpSimd (tensor_mul,
    SBUF-only) to offload DVE.
  - Emission order IS program order: a unit must be emitted after every
    unit it consumes data from. Projection quarter-units interleave into
    the attention stream only where no dependency points backward; exp
    gulp latency is hidden behind them.
  - Both hardware DMA queues used (x/out on sync, w on scalar); phase C
    units split into hp0-6 prefix + hp7 finisher to fill the tail.
"""
import numpy as np

import concourse.bass as bass
import concourse.tile as tile
from concourse import bacc, mybir

F32 = mybir.dt.float32
BF16 = mybir.dt.bfloat16
AF = mybir.ActivationFunctionType

NCORES = 8
B = 2            # batch per core
T = 577
D = 1024
H = 16
E = 64
HE = H * E       # 1024
BT = B * T       # 1154
MT = 8           # m-tiles over HE (head pairs)
KT = 8           # k-tiles over D
VW = E + 1       # 65: augmented V width per head [1 | V]

A_CH = [(0, 385), (385, 385), (770, 384)]                       # proj chunks over BT
TT = [(0, 128), (128, 128), (256, 128), (384, 128), (512, 65)]  # tiles over T (keys)
QN = [(0, 512), (512, 65)]                                      # q chunks
MO = [(0, 128), (128, 128), (256, 128), (384, 128), (512, 65)]  # out-proj m tiles
# qi=1 score tiles: tile j=(2*kt+hd) -> psS bank j%3, col (j//3)*65
# pp column base per gulp, within one (b,hp): see ppcol()


def ppcol(qi, kt, hd):
    if qi == 0:
        return kt * 1024 + hd * 512
    j = 2 * kt + hd
    return 5120 + (j % 2) * 512 + (j // 2) * 65


PP_W = 5120 + 1024  # 6144 bf16 cols per (b,hp)


def build_graph():
    nc = bacc.Bacc("TRN2", target_bir_lowering=False, debug=False,
                   num_devices=NCORES)

    xq = nc.dram_tensor("query_input", [D, BT], BF16, kind="ExternalInput")
    xk = nc.dram_tensor("key_input", [D, BT], BF16, kind="ExternalInput")
    xv = nc.dram_tensor("value_input", [D, BT], BF16, kind="ExternalInput")
    wq = nc.dram_tensor("W_Q", [D, HE], BF16, kind="ExternalInput")
    wk = nc.dram_tensor("W_K", [D, HE], BF16, kind="ExternalInput")
    wv = nc.dram_tensor("W_V", [D, HE], BF16, kind="ExternalInput")
    wo = nc.dram_tensor("W_O", [HE, D], BF16, kind="ExternalInput")
    bq = nc.dram_tensor("b_Q", [128, MT], F32, kind="ExternalInput")
    bk = nc.dram_tensor("b_K", [128, MT], F32, kind="ExternalInput")
    bv = nc.dram_tensor("b_V", [128, MT], F32, kind="ExternalInput")
    bo = nc.dram_tensor("b_O", [1, D], F32, kind="ExternalInput")
    out = nc.dram_tensor("out", [B, T, D], BF16, kind="ExternalOutput")

    with tile.TileContext(nc) as tc:
        _body(nc, tc, xq, xk, xv, wq, wk, wv, wo, bq, bk, bv, bo, out)
    nc.compile()
    return nc


def _body(nc, tc, xq, xk, xv, wq, wk, wv, wo, bq, bk, bv, bo, out):
    from contextlib import ExitStack
    est = ExitStack()
    with est:
        sbQ_p = est.enter_context(tc.tile_pool(name="sbQ", bufs=1))
        sbK_p = est.enter_context(tc.tile_pool(name="sbK", bufs=1))
        sbVg_p = est.enter_context(tc.tile_pool(name="sbVg", bufs=1))
        sbZ_p = est.enter_context(tc.tile_pool(name="sbZ", bufs=1))
        xt_p = est.enter_context(tc.tile_pool(name="xt", bufs=3))
        wt_p = est.enter_context(tc.tile_pool(name="wt", bufs=3))
        pp_p = est.enter_context(tc.tile_pool(name="pp", bufs=2))
        dn_p = est.enter_context(tc.tile_pool(name="dn", bufs=2))
        const_p = est.enter_context(tc.tile_pool(name="const", bufs=1))
        warm_p = est.enter_context(tc.tile_pool(name="warm", bufs=1))
        dram_p = est.enter_context(tc.tile_pool(name="dramd", bufs=1, space="DRAM"))

        bqc = const_p.tile([128, MT], F32, tag="bqc")
        bkc = const_p.tile([128, MT], F32, tag="bkc")
        bvc = const_p.tile([128, MT], F32, tag="bvc")
        boc = const_p.tile([128, D], F32, tag="boc")

        # PE-warmup: DMA boot takes ~9us before any input byte lands. Keep the
        # PE busy on zeros meanwhile so HAM reaches K=8/8 before real work.
        wz = warm_p.tile([128, 512], BF16, tag="wz")
        nc.vector.memset(wz[:], 0.0)
        with tc.tile_pool(name="psW", bufs=2, space="PSUM") as psW_p:
            wps = [psW_p.tile([128, 512], F32, tag="psW", name=f"psW{i}")
                   for i in range(2)]
            for i in range(30):
                nc.tensor.matmul(wps[i % 2][:, :], wz[:, 0:128], wz[:, :],
                                 start=True, stop=True)

        # rings over head-pair m: slot = m % 3
        sbQ = sbQ_p.tile([128, 3 * BT], BF16, tag="sbQ")
        sbK = sbK_p.tile([128, 3 * BT], BF16, tag="sbK")
        sbVg = sbVg_p.tile([128, 10 * H * VW], BF16, tag="sbVg")
        sbZ = sbZ_p.tile([128, B * MT * T], BF16, tag="sbZ")

        def zsl(b, hp, lo, sz, to, tsz):
            base = (b * MT + hp) * T
            return sbZ[lo:lo + sz, base + to:base + to + tsz]

        def load_xw(x_in, w_in, xtag, wtag):
            # interleave x/w chunks across the two HW queues so chunk k of
            # BOTH tensors lands at ~(k+1)/KT of the total transfer time
            xt = xt_p.tile([128, KT * BT], BF16, tag="xt", name=xtag)
            wt = wt_p.tile([128, KT * HE], BF16, tag="wt", name=wtag)
            for k in range(KT):
                ex, ew = (nc.sync, nc.scalar) if k % 2 == 0 else (nc.scalar, nc.sync)
                ex.dma_start(xt[:, k * BT:(k + 1) * BT],
                             x_in.ap()[k * 128:(k + 1) * 128, :])
                ew.dma_start(wt[:, k * HE:(k + 1) * HE],
                             w_in.ap()[k * 128:(k + 1) * 128, :])
            return xt, wt

        # ================= Phase V (first; m0/m1 A-quarters interleave) ======
        psV_ctx = ExitStack()
        psV_p = psV_ctx.enter_context(tc.tile_pool(name="psV", bufs=4, space="PSUM"))
        if True:
            xtv = xt_p.tile([128, KT * BT], BF16, tag="xt", name="xtv")
            wtv = wt_p.tile([128, KT * HE], BF16, tag="wt", name="wtv")
            # k0 split fine so the first matmul's DMA dependency is small
            nc.sync.dma_start(xtv[:, 0:128], xv.ap()[0:128, 0:128])
            nc.scalar.dma_start(wtv[:, 0:512], wv.ap()[0:128, 0:512])
            nc.sync.dma_start(xtv[:, 128:BT], xv.ap()[0:128, 128:BT])
            nc.scalar.dma_start(wtv[:, 512:HE], wv.ap()[0:128, 512:HE])
            nc.gpsimd.dma_start(bqc[:], bq.ap())
            nc.gpsimd.dma_start(bkc[:], bk.ap())
            nc.gpsimd.dma_start(bvc[:], bv.ap())
            nc.gpsimd.dma_start(boc[:], bo.ap().partition_broadcast(128))
            for k in range(1, KT):
                ex, ew = (nc.sync, nc.scalar) if k % 2 == 0 else (nc.scalar, nc.sync)
                ex.dma_start(xtv[:, k * BT:(k + 1) * BT],
                             xv.ap()[k * 128:(k + 1) * 128, :])
                ew.dma_start(wtv[:, k * HE:(k + 1) * HE],
                             wv.ap()[k * 128:(k + 1) * 128, :])

            def v_group(b, ti):
                to, tsz = TT[ti]
                vt = b * 5 + ti
                vbase = vt * H * VW
                bto = b * T + to
                pss = [psV_p.tile([128, 512], F32, tag="psV",
                                  name=f"psV{vt}_{ni}") for ni in range(2)]
                for k in range(KT):
                    for ni in range(2):
                        nc.tensor.matmul(
                            pss[ni][:tsz, :],
                            xtv[:, k * BT + bto:k * BT + bto + tsz],
                            wtv[:, k * HE + ni * 512:k * HE + ni * 512 + 512],
                            start=(k == 0), stop=(k == KT - 1))
                for ni in range(2):
                    dst = sbVg[:tsz, vbase + ni * 8 * VW:
                               vbase + (ni * 8 + 8) * VW].rearrange(
                        "p (h c) -> p h c", c=VW)[:, :, 0:E]
                    src = pss[ni][:tsz, :].rearrange(
                        "p (h c) -> p h c", c=E)
                    nc.vector.tensor_copy(dst, src)
                onecols = sbVg[:tsz, vbase:vbase + H * VW].rearrange(
                    "p (h c) -> p h c", c=VW)[:, :, E:E + 1]
                nc.vector.memset(onecols, 1.0)

            for ti in range(5):
                v_group(0, ti)
            # b1 groups emitted below, interleaved with early A-quarters

        xtq, wtq = load_xw(xq, wq, "xtq", "wtq")
        xtk, wtk = load_xw(xk, wk, "xtk", "wtk")

        # ================= m-loop: A (Q/K proj) interleaved with B ========
        # psA opens before phase V's b1 half so m0 quarters can interleave
        pa = ExitStack()
        psA_p = pa.enter_context(tc.tile_pool(name="psA", bufs=2, space="PSUM",
                                              side="right"))

        # ---- emitted-PE-work clock (ns) for evacuation-cover gating ----
        pe_ns = [0.0]
        h3_mark = [-1e9]
        GATE_NS = 650.0  # measured best; 550 and 800 both slower

        # ---- A units ----
        a_queue = []

        def make_a_units(m):
            # 6 chunks: (proj, chunk) pairs -> 3 pairs. Each pair becomes 5
            # fill units: h0-h2 (4 MMs each), h3 (j0k6,j1k6,j0k7 -- tile j0
            # finishes early and its DVE evacuation overlaps the closer +
            # interleaved attention), closer (j1k7 + j1's evacuation). This
            # staggers the two serial DVE evacuations so neither sits on the
            # next pair's PSUM-reuse critical path.
            chunks = [(xtq, wtq, bqc, sbQ, ci) for ci in range(3)] + \
                     [(xtk, wtk, bkc, sbK, ci) for ci in range(3)]
            for pi in range(3):
                pair = chunks[2 * pi:2 * pi + 2]
                tiles = [psA_p.tile([128, 385], F32, tag="psA",
                                    name=f"psA_m{m}_{pi}_{j}")
                         for j in range(2)]

                def mk(ops, pair=pair, tiles=tiles, m=m):
                    def unit():
                        r = m % 3
                        for j, k in ops:
                            xt, wt, bc, dst, ci = pair[j]
                            co, csz = A_CH[ci]
                            nc.tensor.matmul(
                                tiles[j][:, :csz],
                                wt[:, k * HE + m * 128:k * HE + (m + 1) * 128],
                                xt[:, k * BT + co:k * BT + co + csz],
                                start=(k == 0), stop=(k == KT - 1))
                        pe_ns[0] += 163.0 * len(ops)
                        for j, k in ops:
                            if k == KT - 1:
                                xt, wt, bc, dst, ci = pair[j]
                                co, csz = A_CH[ci]
                                nc.vector.tensor_scalar_add(
                                    dst[:, r * BT + co:r * BT + co + csz],
                                    tiles[j][:, :csz], bc[:, m:m + 1])
                                h3_mark[0] = pe_ns[0]
                    return unit

                seq = [[(0, 2 * h), (1, 2 * h), (0, 2 * h + 1), (1, 2 * h + 1)]
                       for h in range(3)]
                seq.append([(0, 6), (1, 6), (0, 7)])
                seq.append([(1, 7)])
                for idx, ops in enumerate(seq):
                    a_queue.append((idx, mk(ops)))

        def fill(n=1, force=False):
            # pop up to n quarters; a pair's first quarter (h0) is deferred
            # until >=GATE_NS of PE work covers the previous pair's PSUM
            # evacuation (DVE add) -- unless forced.
            for _ in range(n):
                if not a_queue:
                    return
                h, fn = a_queue[0]
                if (not force and h == 0
                        and pe_ns[0] - h3_mark[0] < GATE_NS):
                    return
                a_queue.pop(0)
                fn()

        def make_filler(budget):
            box = [budget]

            def bf(n=2):
                take = min(n, box[0], len(a_queue))
                before = len(a_queue)
                fill(take)
                box[0] -= before - len(a_queue)
            return bf

        # ---- phase V b1 half, interleaved with m0 A-quarters ----
        make_a_units(0)
        for ti in range(5):
            v_group(1, ti)
            pe_ns[0] += 3453.0
            if ti >= 1:
                fill(2)
        fill(len(a_queue), force=True)  # m0 must complete before B(0,0)
        psV_ctx.close()

        bs = ExitStack()
        psS_p = bs.enter_context(tc.tile_pool(name="psS", bufs=2, space="PSUM"))
        psZ_p = bs.enter_context(tc.tile_pool(name="psZ", bufs=2, space="PSUM"))

        # ---- B emission for one (b, hp) ----
        def emit_bhp(b, hp, fills=fill):
            r = hp % 3
            qb = b * T
            pp = pp_p.tile([128, PP_W], BF16, tag="pp", name=f"pp{b}_{hp}")
            # denominators for both heads live in partition 0 (cols hd*T..):
            # gpsimd.partition_broadcast only honors base partition 0 APs
            ddf = dn_p.tile([1, 2 * T], F32, tag="ddf", name=f"ddf{b}_{hp}")
            ddb = dn_p.tile([1, 2 * T], BF16, tag="ddb", name=f"ddb{b}_{hp}")
            psz = {}

            def sc(qi, kt):
                # one fresh 2-bank psS tile per (qi0, kt); one tile for all qi1
                st = psS_p.tile([128, 1024], F32, tag="psS",
                                name=f"psS{b}_{hp}_{qi}_{kt}")
                qo, qsz = QN[qi]
                if qi == 0:
                    ko, ksz = TT[kt]
                    for hd in range(2):
                        lo = hd * 64
                        nc.tensor.matmul(
                            st[:ksz, hd * 512:hd * 512 + qsz],
                            sbK[lo:lo + 64, r * BT + qb + ko:r * BT + qb + ko + ksz],
                            sbQ[lo:lo + 64, r * BT + qb + qo:r * BT + qb + qo + qsz],
                            start=True, stop=True, tile_position=(lo, 0))
                    pe_ns[0] += 218.0
                else:
                    for kt2 in range(5):
                        ko, ksz = TT[kt2]
                        for hd in range(2):
                            lo = hd * 64
                            j = 2 * kt2 + hd
                            dcol = (j % 2) * 512 + (j // 2) * 65
                            nc.tensor.matmul(
                                st[:ksz, dcol:dcol + qsz],
                                sbK[lo:lo + 64, r * BT + qb + ko:r * BT + qb + ko + ksz],
                                sbQ[lo:lo + 64, r * BT + qb + qo:r * BT + qb + qo + qsz],
                                start=True, stop=True, tile_position=(lo, 0))
                    pe_ns[0] += 160.0
                # flat full-tile exp gulp: write-once-read-once per generation
                # (per-hd splitting measured slower even in fill-less regions:
                # ACT is serial, so the last-needed-byte latency is unchanged
                # while the extra instruction overhead adds up)
                rows = 128 if qi == 1 or TT[kt][1] == 128 else 65
                base = kt * 1024 if qi == 0 else 5120
                nc.scalar.activation(pp[:rows, base:base + 1024],
                                     st[:rows, :], AF.Exp, scale=0.125)

            def av(qi, kts):
                qo, qsz = QN[qi]
                for kt in kts:
                    ko, ksz = TT[kt]
                    vbase = (b * 5 + kt) * H * VW
                    for hd in range(2):
                        h = 2 * hp + hd
                        if kt == 0:
                            psz[(qi, hd)] = psZ_p.tile(
                                [65, 512], F32, tag="psZ",
                                name=f"psZ{b}_{hp}_{qi}_{hd}")
                        nc.tensor.matmul(
                            psz[(qi, hd)][:, :qsz],
                            sbVg[:ksz, vbase + h * VW:vbase + h * VW + VW],
                            pp[:ksz, ppcol(qi, kt, hd):ppcol(qi, kt, hd) + qsz],
                            start=(kt == 0), stop=(kt == 4))
                    pe_ns[0] += 2 * (qsz / 2.4 + 2.5)

            def finz(qi):
                qo, qsz = QN[qi]
                for hd in range(2):
                    lo = hd * 64
                    nc.vector.tensor_scalar_add(
                        zsl(b, hp, lo, 64, qo, qsz),
                        psz[(qi, hd)][0:64, :qsz],
                        bvc[lo:lo + 64, hp:hp + 1])
                    nc.vector.tensor_copy(
                        ddf[0:1, hd * T + qo:hd * T + qo + qsz],
                        psz[(qi, hd)][64:65, :qsz])

            # ---- emission sequence (ACT-paced: one fill per kt) ----
            sc(0, 0)
            sc(0, 1)
            fills()
            av(0, [0])
            sc(0, 2)
            fills()
            av(0, [1])
            sc(0, 3)
            fills()
            av(0, [2])
            sc(0, 4)
            fills()
            av(0, [3])
            av(0, [4])
            finz(0)
            sc(1, 0)
            fills()
            av(1, range(5))
            finz(1)
            # normalize: broadcast 1/denom via DRAM roundtrip, then scale z
            nc.vector.reciprocal_approx_fast(ddf[:], ddf[:])
            nc.vector.tensor_copy(ddb[:], ddf[:])
            dd = dram_p.tile([1, 2 * T], BF16, tag=f"dd{b}_{hp}",
                             name=f"dd{b}_{hp}")
            rb = dn_p.tile([128, T], BF16, tag="rb", name=f"rb{b}_{hp}")
            nc.sync.dma_start(dd[0:1, :], ddb[0:1, :])
            for hd in range(2):
                lo = hd * 64
                nc.sync.dma_start(
                    rb[lo:lo + 64, :],
                    dd[0:1, hd * T:hd * T + T].partition_broadcast(64))
                nc.gpsimd.tensor_mul(zsl(b, hp, lo, 64, 0, T),
                                     zsl(b, hp, lo, 64, 0, T), rb[lo:lo + 64, :])
            fills()

        # ---- the m-loop ----
        # W_O prefetch: reuses wv's buffer slot, DMA starts once V-phase is
        # done. Kept OFF the scalar engine: its DMA_DIRECT2D triggers cost
        # ~605ns of ACT time each and head-block exp gulps behind the
        # wtv-WAR semaphore right as the m-loop starts (ACT is the m-loop
        # pacer at ~87% busy).
        wot = wt_p.tile([128, MT * D], BF16, tag="wt", name="wot")
        for hp in range(MT):
            eng = nc.sync if hp % 2 == 0 else nc.gpsimd
            eng.dma_start(wot[:, hp * D:(hp + 1) * D],
                          wo.ap()[hp * 128:(hp + 1) * 128, :])
        for m in range(1, MT):
            make_a_units(m)
            emit_bhp(0, m - 1, fills=make_filler(8))
            emit_bhp(1, m - 1, fills=make_filler(8))
            fill(len(a_queue), force=True)  # drain leftovers to stay ahead
        # tail: last head pair; phase C interleaves below
        # ================= Phase C =================
        pa.close()  # frees psA (2 banks) before psO opens

        sbO_p = est.enter_context(tc.tile_pool(name="sbO", bufs=3))
        co = ExitStack()
        psO_p = co.enter_context(tc.tile_pool(name="psO", bufs=2, space="PSUM"))

        c_open = []  # dicts {b, mi, tiles, hp}
        c_queue = [(b, mi) for b in range(B) for mi in range(5)]

        def c_start(pool=None):
            pool = pool or psO_p
            b, mi = c_queue.pop(0)
            tiles = [pool.tile([128, 512], F32, tag="psO",
                               name=f"psO{b}_{mi}_{ni}") for ni in range(2)]
            c_open.append({"b": b, "mi": mi, "tiles": tiles, "hp": 0})

        def c_advance(nhp):
            u = c_open[-1]
            mo, msz = MO[u["mi"]]
            end = min(u["hp"] + nhp, MT - 1)
            for hp in range(u["hp"], end):
                for ni in range(2):
                    nc.tensor.matmul(
                        u["tiles"][ni][:msz, :],
                        zsl(u["b"], hp, 0, 128, mo, msz),
                        wot[:, hp * D + ni * 512:hp * D + ni * 512 + 512],
                        start=(hp == 0), stop=False)
                pe_ns[0] += 432.0
            u["hp"] = end

        def c_finish():
            u = c_open.pop(0)
            b, mi = u["b"], u["mi"]
            mo, msz = MO[mi]
            while u["hp"] < MT - 1:  # complete any unfinished prefix
                hp = u["hp"]
                for ni in range(2):
                    nc.tensor.matmul(
                        u["tiles"][ni][:msz, :],
                        zsl(b, hp, 0, 128, mo, msz),
                        wot[:, hp * D + ni * 512:hp * D + ni * 512 + 512],
                        start=(hp == 0), stop=False)
                pe_ns[0] += 432.0
                u["hp"] = hp + 1
            hp = MT - 1
            for ni in range(2):
                nc.tensor.matmul(
                    u["tiles"][ni][:msz, :],
                    zsl(b, hp, 0, 128, mo, msz),
                    wot[:, hp * D + ni * 512:hp * D + ni * 512 + 512],
                    start=False, stop=True)
            pe_ns[0] += 432.0
            for ni in range(2):
                so = sbO_p.tile([128, 512], BF16, tag="sbO",
                                name=f"sbO{b}_{mi}_{ni}")
                nc.vector.tensor_add(so[:msz, :], u["tiles"][ni][:msz, :],
                                     boc[:msz, ni * 512:ni * 512 + 512])
                eng = nc.sync if ni == 0 else nc.scalar
                eng.dma_start(
                    out.ap()[b, mo:mo + msz, ni * 512:ni * 512 + 512],
                    so[:msz, :])

        def cfill_b0(n=1):
            # during emit_bhp(0,7): prefix ONE b0 unit, granularly (3 hp/call).
            # A second open unit would WAR on the same psO banks.
            if not c_open and c_queue and c_queue[0][0] == 0:
                c_start()
            if c_open and c_open[-1]["hp"] < MT - 1:
                c_advance(3)

        b0_used = [0]

        def cfill_b1(n=1):
            # during emit_bhp(1,7): z(b0) fully ready. Alternate finish/open,
            # but keep >=2 b0 units in reserve to cover the b1-hp7 norm tail.
            if c_open and c_open[0]["hp"] >= MT - 1:
                c_finish()
            elif c_open and c_open[-1]["hp"] < MT - 1:
                c_advance(4)
            elif (not c_open and c_queue and c_queue[0][0] == 0
                  and b0_used[0] < 3):
                b0_used[0] += 1
                c_start()

        emit_bhp(0, MT - 1, fills=cfill_b0)
        emit_bhp(1, MT - 1, fills=cfill_b1)
        while c_open:
            c_finish()
        co.close()
        bs.close()
        psO3_p = est.enter_context(tc.tile_pool(name="psO3", bufs=4, space="PSUM"))
        while c_queue or c_open:
            if c_queue:
                c_start(pool=psO3_p)
                c_advance(MT - 1)
            if len(c_open) >= 2 or not c_queue:
                c_finish()


_GRAPH = None


def _get_graph():
    global _GRAPH
    if _GRAPH is None:
        _GRAPH = build_graph()
    return _GRAPH


def kernel(query_input, key_input, value_input, W_Q, W_K, W_V, W_O,
           b_Q, b_K, b_V, b_O, _trace=False, _trace_kwargs=None):
    import ml_dtypes
    from concourse.bass_utils import run_bass_kernel_spmd

    nc = _get_graph()
    f = np.ascontiguousarray
    bf = ml_dtypes.bfloat16

    def xT(x, sl):
        x = np.asarray(x[sl], np.float32)
        return f(x.reshape(B * T, D).T.astype(bf))

    def wT(w):
        w = np.asarray(w, np.float32)
        return f(w.transpose(1, 0, 2).reshape(D, HE).astype(bf))

    def bcol(bx):
        bx = np.asarray(bx, np.float32).reshape(HE)
        return f(bx.reshape(MT, 128).T)

    wq_m, wk_m, wv_m = wT(W_Q), wT(W_K), wT(W_V)
    wo_m = f(np.asarray(W_O, np.float32).reshape(HE, D).astype(bf))
    bq_m, bk_m, bv_m = bcol(b_Q), bcol(b_K), bcol(b_V)
    bo_m = f(np.asarray(b_O, np.float32).reshape(1, D))
    in_maps = []
    for c in range(NCORES):
        sl = slice(2 * c, 2 * c + 2)
        in_maps.append({
            "query_input": xT(query_input, sl),
            "key_input": xT(key_input, sl),
            "value_input": xT(value_input, sl),
            "W_Q": wq_m,
            "W_K": wk_m,
            "W_V": wv_m,
            "W_O": wo_m,
            "b_Q": bq_m,
            "b_K": bk_m,
            "b_V": bv_m,
            "b_O": bo_m,
        })
    res = run_bass_kernel_spmd(nc, in_maps, core_ids=list(range(NCORES)),
                               trace=_trace, **(_trace_kwargs or {}))
    outp = np.concatenate([np.asarray(res.results[c]["out"], np.float32)
                           for c in range(NCORES)], axis=0)
    if _trace:
        kernel._last_result = res
    return outp



# revision 48
# speedup vs baseline: 1.1828x; 1.1828x over previous
"""Multi-head attention kernel for Trainium2, 8 NeuronCores, data-parallel over batch.

Problem: batch=16, pos=577, d_model=1024, n_heads=16, d_head=64, fp32.
Sharding: batch across 8 cores (2 batch items per core), no collectives.

v17 (~259.3us HW at the chip's normal 2.4GHz power state; v13 ~260-261us,
v9 ~262-265us, earlier baseline 307us). NOTE: under sustained benching the
chip drops into P0 (PE ~2.0GHz, N=512 MM spacing 259ns instead of 216ns)
and the same kernel measures ~304-309us; ~2-3min of idle restores 2.4GHz.
Always check MM spacing in the trace before comparing runs. Key mechanics,
all trace-verified:
  - PE streams moving columns at 2.4GHz (measured: N=512 MMs at 216ns
    start-to-start = 512/2.4+2.5, N=385 at 163ns). Issue-cycle floor for
    this kernel is ~450k cols ~ 187us; consecutive matmuls must target
    different PSUM banks (else +drain barrier) -> every accumulation chain
    alternates banks between consecutive MMs.
  - v17 adds over v13: each A-pair's two PSUM accumulation chains are
    STAGGERED (tile j0 ends its k-chain one slot early; its DVE
    evacuation overlaps the j1k7 closer MM + one attention block), so the
    two serial DVE evacuations stop sitting on the next pair's PSUM-reuse
    critical path. Pairs are 5 fill units (h0-h2: 4 MMs, h3: j0k6/j1k6/
    j0k7 + evac0, closer: j1k7 + evac1); filler budget 8/emit, 2 pops max
    per site, pair-start gate at 650ns (550 and 800 both measured slower).
  - v13 adds over v9: (1) ~30 zero-input warmup MMs at t=0 (the DMA
    descriptor-table boot means no input byte lands before ~9us; warmups
    get HAM to K=8/8 early), (2) x/w DMA chunks interleaved across both HW
    queues so chunk k of both tensors lands at ~(k+1)/8 of the transfer,
    (3) m0's Q/K-proj quarters emitted interleaved into phase V's b1 half
    (B(0,0) starts ~8us earlier; pair boundaries covered by V MMs),
    (4) granular phase-C fills (per-3-hp prefix advance) + reserve b0
    units so the b1-hp7 normalization latency is covered by PE work,
    (5) out-writes alternate sync/scalar, wo prefetch split across both.
  - Measured dead ends (do NOT revisit): per-hd exp splitting (+20us ACT
    busy, exps are the attention pacer -> net loss); A-evac on ACT via
    activation(Identity, bias=...) (delays exps in the strict ACT FIFO);
    gpsimd.partition_broadcast for the denominator broadcast (1.1us/instr
    ucode cost + cross-engine ping-pong, net +24us; also its src AND dst
    APs only honor base partition 0); warm filler MMs between V groups
    (+4us); outs on gpsimd SWDGE (slow tail drain).
  - The dd/rb DMA roundtrip sits ~30-50us behind the input stream on the
    sync queue mid-loop (queue FIFO); harmless for the m-loop (z-scale is
    only needed by phase C) but it is why phase C can stall ~5us at
    ~210us. Known remaining stalls: ~25us spread through the m-loop
    (psA evacuation WAR: next pair's first MM waits sem+DVE-add+sem
    ~714ns; interleaved attention covers only ~645ns), ~8us teardown tail.
  - Score matmuls for the two heads of a pair are emitted adjacently with
    row-disjoint tile_position (K=64 each) -> concurrent in the PE array,
    ~2x effective rate.
  - q-chunks (512, 65). Scores for one kt land in a fresh 2-bank psS pool
    tile ([kszx512 | kszx512] h0|h1); exp is ONE flat [ksz,1024] ACT gulp
    per tile (352-cyc fixed cost amortized; PSUM regions are strictly
    write-once-read-once per pool-tile generation, which the dep tracker
    handles robustly).
  - Augmented V = [V_h | 1]: AV matmul row 64 = softmax denominator for
    free. Denominator rows are copied out with plain tensor_copy (NOTE:
    reciprocal_approx_fast reads the wrong partition when src/dst base
    partitions differ - copy first, then recip in place), reciprocal'd
    batched, broadcast via DRAM roundtrip, z scaled on GpSimd (tensor_mul,
    SBUF-only) to offload DVE.
  - Emission order IS program order: a unit must be emitted after every
    unit it consumes data from. Projection quarter-units interleave into
    the attention stream only where no dependency points backward; exp
    gulp latency is hidden behind them.
  - Both hardware DMA queues used (x/out on sync, w on scalar); phase C
    units split into hp0-6 prefix + hp7 finisher to fill the tail.
"""
import numpy as np

import concourse.bass as bass
import concourse.tile as tile
from concourse import bacc, mybir

F32 = mybir.dt.float32
BF16 = mybir.dt.bfloat16
AF = mybir.ActivationFunctionType

NCORES = 8
B = 2            # batch per core
T = 577
D = 1024
H = 16
E = 64
HE = H * E       # 1024
BT = B * T       # 1154
MT = 8           # m-tiles over HE (head pairs)
KT = 8           # k-tiles over D
VW = E + 1       # 65: augmented V width per head [1 | V]

A_CH = [(0, 385), (385, 385), (770, 384)]                       # proj chunks over BT
TT = [(0, 128), (128, 128), (256, 128), (384, 128), (512, 65)]  # tiles over T (keys)
QN = [(0, 512), (512, 65)]                                      # q chunks
MO = [(0, 128), (128, 128), (256, 128), (384, 128), (512, 65)]  # out-proj m tiles
# qi=1 score tiles: tile j=(2*kt+hd) -> psS bank j%3, col (j//3)*65
# pp column base per gulp, within one (b,hp): see ppcol()


def ppcol(qi, kt, hd):
    if qi == 0:
        return kt * 1024 + hd * 512
    j = 2 * kt + hd
    return 5120 + (j % 2) * 512 + (j // 2) * 65


PP_W = 5120 + 1024  # 6144 bf16 cols per (b,hp)


def build_graph():
    nc = bacc.Bacc("TRN2", target_bir_lowering=False, debug=False,
                   num_devices=NCORES)

    xq = nc.dram_tensor("query_input", [D, BT], BF16, kind="ExternalInput")
    xk = nc.dram_tensor("key_input", [D, BT], BF16, kind="ExternalInput")
    xv = nc.dram_tensor("value_input", [D, BT], BF16, kind="ExternalInput")
    wq = nc.dram_tensor("W_Q", [D, HE], BF16, kind="ExternalInput")
    wk = nc.dram_tensor("W_K", [D, HE], BF16, kind="ExternalInput")
    wv = nc.dram_tensor("W_V", [D, HE], BF16, kind="ExternalInput")
    wo = nc.dram_tensor("W_O", [HE, D], BF16, kind="ExternalInput")
    bq = nc.dram_tensor("b_Q", [128, MT], F32, kind="ExternalInput")
    bk = nc.dram_tensor("b_K", [128, MT], F32, kind="ExternalInput")
    bv = nc.dram_tensor("b_V", [128, MT], F32, kind="ExternalInput")
    bo = nc.dram_tensor("b_O", [1, D], F32, kind="ExternalInput")
    out = nc.dram_tensor("out", [B, T, D], BF16, kind="ExternalOutput")

    with tile.TileContext(nc) as tc:
        _body(nc, tc, xq, xk, xv, wq, wk, wv, wo, bq, bk, bv, bo, out)
    nc.compile()
    return nc


def _body(nc, tc, xq, xk, xv, wq, wk, wv, wo, bq, bk, bv, bo, out):
    from contextlib import ExitStack
    est = ExitStack()
    with est:
        sbQ_p = est.enter_context(tc.tile_pool(name="sbQ", bufs=1))
        sbK_p = est.enter_context(tc.tile_pool(name="sbK", bufs=1))
        sbVg_p = est.enter_context(tc.tile_pool(name="sbVg", bufs=1))
        sbZ_p = est.enter_context(tc.tile_pool(name="sbZ", bufs=1))
        xt_p = est.enter_context(tc.tile_pool(name="xt", bufs=3))
        wt_p = est.enter_context(tc.tile_pool(name="wt", bufs=3))
        pp_p = est.enter_context(tc.tile_pool(name="pp", bufs=2))
        dn_p = est.enter_context(tc.tile_pool(name="dn", bufs=2))
        const_p = est.enter_context(tc.tile_pool(name="const", bufs=1))
        warm_p = est.enter_context(tc.tile_pool(name="warm", bufs=1))
        dram_p = est.enter_context(tc.tile_pool(name="dramd", bufs=1, space="DRAM"))

        bqc = const_p.tile([128, MT], F32, tag="bqc")
        bkc = const_p.tile([128, MT], F32, tag="bkc")
        bvc = const_p.tile([128, MT], F32, tag="bvc")
        boc = const_p.tile([128, D], F32, tag="boc")

        # PE-warmup: DMA boot takes ~9us before any input byte lands. Keep the
        # PE busy on zeros meanwhile so HAM reaches K=8/8 before real work.
        wz = warm_p.tile([128, 512], BF16, tag="wz")
        nc.vector.memset(wz[:], 0.0)
        with tc.tile_pool(name="psW", bufs=2, space="PSUM") as psW_p:
            wps = [psW_p.tile([128, 512], F32, tag="psW", name=f"psW{i}")
                   for i in range(2)]
            for i in range(30):
                nc.tensor.matmul(wps[i % 2][:, :], wz[:, 0:128], wz[:, :],
                                 start=True, stop=True)

        # rings over head-pair m: slot = m % 3
        sbQ = sbQ_p.tile([128, 3 * BT], BF16, tag="sbQ")
        sbK = sbK_p.tile([128, 3 * BT], BF16, tag="sbK")
        sbVg = sbVg_p.tile([128, 10 * H * VW], BF16, tag="sbVg")
        sbZ = sbZ_p.tile([128, B * MT * T], BF16, tag="sbZ")

        def zsl(b, hp, lo, sz, to, tsz):
            base = (b * MT + hp) * T
            return sbZ[lo:lo + sz, base + to:base + to + tsz]

        def load_xw(x_in, w_in, xtag, wtag):
            # interleave x/w chunks across the two HW queues so chunk k of
            # BOTH tensors lands at ~(k+1)/KT of the total transfer time
            xt = xt_p.tile([128, KT * BT], BF16, tag="xt", name=xtag)
            wt = wt_p.tile([128, KT * HE], BF16, tag="wt", name=wtag)
            for k in range(KT):
                ex, ew = (nc.sync, nc.scalar) if k % 2 == 0 else (nc.scalar, nc.sync)
                ex.dma_start(xt[:, k * BT:(k + 1) * BT],
                             x_in.ap()[k * 128:(k + 1) * 128, :])
                ew.dma_start(wt[:, k * HE:(k + 1) * HE],
                             w_in.ap()[k * 128:(k + 1) * 128, :])
            return xt, wt

        # ================= Phase V (first; m0/m1 A-quarters interleave) ======
        psV_ctx = ExitStack()
        psV_p = psV_ctx.enter_context(tc.tile_pool(name="psV", bufs=4, space="PSUM"))
        if True:
            xtv = xt_p.tile([128, KT * BT], BF16, tag="xt", name="xtv")
            wtv = wt_p.tile([128, KT * HE], BF16, tag="wt", name="wtv")
            # k0 split fine so the first matmul's DMA dependency is small
            nc.sync.dma_start(xtv[:, 0:128], xv.ap()[0:128, 0:128])
            nc.scalar.dma_start(wtv[:, 0:512], wv.ap()[0:128, 0:512])
            nc.sync.dma_start(xtv[:, 128:BT], xv.ap()[0:128, 128:BT])
            nc.scalar.dma_start(wtv[:, 512:HE], wv.ap()[0:128, 512:HE])
            nc.gpsimd.dma_start(bqc[:], bq.ap())
            nc.gpsimd.dma_start(bkc[:], bk.ap())
            nc.gpsimd.dma_start(bvc[:], bv.ap())
            nc.gpsimd.dma_start(boc[:], bo.ap().partition_broadcast(128))
            for k in range(1, KT):
                ex, ew = (nc.sync, nc.scalar) if k % 2 == 0 else (nc.scalar, nc.sync)
                ex.dma_start(xtv[:, k * BT:(k + 1) * BT],
                             xv.ap()[k * 128:(k + 1) * 128, :])
                ew.dma_start(wtv[:, k * HE:(k + 1) * HE],
                             wv.ap()[k * 128:(k + 1) * 128, :])

            def v_group(b, ti):
                to, tsz = TT[ti]
                vt = b * 5 + ti
                vbase = vt * H * VW
                bto = b * T + to
                pss = [psV_p.tile([128, 512], F32, tag="psV",
                                  name=f"psV{vt}_{ni}") for ni in range(2)]
                for k in range(KT):
                    for ni in range(2):
                        nc.tensor.matmul(
                            pss[ni][:tsz, :],
                            xtv[:, k * BT + bto:k * BT + bto + tsz],
                            wtv[:, k * HE + ni * 512:k * HE + ni * 512 + 512],
                            start=(k == 0), stop=(k == KT - 1))
                for ni in range(2):
                    dst = sbVg[:tsz, vbase + ni * 8 * VW:
                               vbase + (ni * 8 + 8) * VW].rearrange(
                        "p (h c) -> p h c", c=VW)[:, :, 0:E]
                    src = pss[ni][:tsz, :].rearrange(
                        "p (h c) -> p h c", c=E)
                    nc.vector.tensor_copy(dst, src)
                onecols = sbVg[:tsz, vbase:vbase + H * VW].rearrange(
                    "p (h c) -> p h c", c=VW)[:, :, E:E + 1]
                nc.vector.memset(onecols, 1.0)

            for ti in range(5):
                v_group(0, ti)
            # b1 groups emitted below, interleaved with early A-quarters

        xtq, wtq = load_xw(xq, wq, "xtq", "wtq")
        xtk, wtk = load_xw(xk, wk, "xtk", "wtk")

        # ================= m-loop: A (Q/K proj) interleaved with B ========
        # psA opens before phase V's b1 half so m0 quarters can interleave
        pa = ExitStack()
        psA_p = pa.enter_context(tc.tile_pool(name="psA", bufs=2, space="PSUM",
                                              side="right"))

        # ---- emitted-PE-work clock (ns) for evacuation-cover gating ----
        pe_ns = [0.0]
        h3_mark = [-1e9]
        GATE_NS = 650.0  # measured best; 550 and 800 both slower

        # ---- A units ----
        a_queue = []

        def make_a_units(m):
            # 6 chunks: (proj, chunk) pairs -> 3 pairs. Each pair becomes 5
            # fill units: h0-h2 (4 MMs each), h3 (j0k6,j1k6,j0k7 -- tile j0
            # finishes early and its DVE evacuation overlaps the closer +
            # interleaved attention), closer (j1k7 + j1's evacuation). This
            # staggers the two serial DVE evacuations so neither sits on the
            # next pair's PSUM-reuse critical path.
            chunks = [(xtq, wtq, bqc, sbQ, ci) for ci in range(3)] + \
                     [(xtk, wtk, bkc, sbK, ci) for ci in range(3)]
            for pi in range(3):
                pair = chunks[2 * pi:2 * pi + 2]
                tiles = [psA_p.tile([128, 385], F32, tag="psA",
                                    name=f"psA_m{m}_{pi}_{j}")
                         for j in range(2)]

                def mk(ops, pair=pair, tiles=tiles, m=m):
                    def unit():
                        r = m % 3
                        for j, k in ops:
                            xt, wt, bc, dst, ci = pair[j]
                            co, csz = A_CH[ci]
                            nc.tensor.matmul(
                                tiles[j][:, :csz],
                                wt[:, k * HE + m * 128:k * HE + (m + 1) * 128],
                                xt[:, k * BT + co:k * BT + co + csz],
                                start=(k == 0), stop=(k == KT - 1))
                        pe_ns[0] += 163.0 * len(ops)
                        for j, k in ops:
                            if k == KT - 1:
                                xt, wt, bc, dst, ci = pair[j]
                                co, csz = A_CH[ci]
                                nc.vector.tensor_scalar_add(
                                    dst[:, r * BT + co:r * BT + co + csz],
                                    tiles[j][:, :csz], bc[:, m:m + 1])
                                h3_mark[0] = pe_ns[0]
                    return unit

                seq = [[(0, 2 * h), (1, 2 * h), (0, 2 * h + 1), (1, 2 * h + 1)]
                       for h in range(3)]
                seq.append([(0, 6), (1, 6), (0, 7)])
                seq.append([(1, 7)])
                for idx, ops in enumerate(seq):
                    a_queue.append((idx, mk(ops)))

        def fill(n=1, force=False):
            # pop up to n quarters; a pair's first quarter (h0) is deferred
            # until >=GATE_NS of PE work covers the previous pair's PSUM
            # evacuation (DVE add) -- unless forced.
            for _ in range(n):
                if not a_queue:
                    return
                h, fn = a_queue[0]
                if (not force and h == 0
                        and pe_ns[0] - h3_mark[0] < GATE_NS):
                    return
                a_queue.pop(0)
                fn()

        def make_filler(budget):
            box = [budget]

            def bf(n=2):
                take = min(n, box[0], len(a_queue))
                before = len(a_queue)
                fill(take)
                box[0] -= before - len(a_queue)
            return bf

        # ---- phase V b1 half, interleaved with m0 A-quarters ----
        make_a_units(0)
        for ti in range(5):
            v_group(1, ti)
            pe_ns[0] += 3453.0
            if ti >= 1:
                fill(2)
        fill(len(a_queue), force=True)  # m0 must complete before B(0,0)
        psV_ctx.close()

        bs = ExitStack()
        psS_p = bs.enter_context(tc.tile_pool(name="psS", bufs=2, space="PSUM"))
        psZ_p = bs.enter_context(tc.tile_pool(name="psZ", bufs=2, space="PSUM"))

        # ---- B emission for one (b, hp) ----
        def emit_bhp(b, hp, fills=fill):
            r = hp % 3
            qb = b * T
            pp = pp_p.tile([128, PP_W], BF16, tag="pp", name=f"pp{b}_{hp}")
            # denominators for both heads live in partition 0 (cols hd*T..):
            # gpsimd.partition_broadcast only honors base partition 0 APs
            ddf = dn_p.tile([1, 2 * T], F32, tag="ddf", name=f"ddf{b}_{hp}")
            ddb = dn_p.tile([1, 2 * T], BF16, tag="ddb", name=f"ddb{b}_{hp}")
            psz = {}

            def sc(qi, kt):
                # one fresh 2-bank psS tile per (qi0, kt); one tile for all qi1
                st = psS_p.tile([128, 1024], F32, tag="psS",
                                name=f"psS{b}_{hp}_{qi}_{kt}")
                qo, qsz = QN[qi]
                if qi == 0:
                    ko, ksz = TT[kt]
                    for hd in range(2):
                        lo = hd * 64
                        nc.tensor.matmul(
                            st[:ksz, hd * 512:hd * 512 + qsz],
                            sbK[lo:lo + 64, r * BT + qb + ko:r * BT + qb + ko + ksz],
                            sbQ[lo:lo + 64, r * BT + qb + qo:r * BT + qb + qo + qsz],
                            start=True, stop=True, tile_position=(lo, 0))
                    pe_ns[0] += 218.0
                else:
                    for kt2 in range(5):
                        ko, ksz = TT[kt2]
                        for hd in range(2):
                            lo = hd * 64
                            j = 2 * kt2 + hd
                            dcol = (j % 2) * 512 + (j // 2) * 65
                            nc.tensor.matmul(
                                st[:ksz, dcol:dcol + qsz],
                                sbK[lo:lo + 64, r * BT + qb + ko:r * BT + qb + ko + ksz],
                                sbQ[lo:lo + 64, r * BT + qb + qo:r * BT + qb + qo + qsz],
                                start=True, stop=True, tile_position=(lo, 0))
                    pe_ns[0] += 160.0
                # flat full-tile exp gulp: write-once-read-once per generation
                # (per-hd splitting measured slower even in fill-less regions:
                # ACT is serial, so the last-needed-byte latency is unchanged
                # while the extra instruction overhead adds up)
                rows = 128 if qi == 1 or TT[kt][1] == 128 else 65
                base = kt * 1024 if qi == 0 else 5120
                nc.scalar.activation(pp[:rows, base:base + 1024],
                                     st[:rows, :], AF.Exp, scale=0.125)

            def av(qi, kts):
                qo, qsz = QN[qi]
                for kt in kts:
                    ko, ksz = TT[kt]
                    vbase = (b * 5 + kt) * H * VW
                    for hd in range(2):
                        h = 2 * hp + hd
                        if kt == 0:
                            psz[(qi, hd)] = psZ_p.tile(
                                [65, 512], F32, tag="psZ",
                                name=f"psZ{b}_{hp}_{qi}_{hd}")
                        nc.tensor.matmul(
                            psz[(qi, hd)][:, :qsz],
                            sbVg[:ksz, vbase + h * VW:vbase + h * VW + VW],
                            pp[:ksz, ppcol(qi, kt, hd):ppcol(qi, kt, hd) + qsz],
                            start=(kt == 0), stop=(kt == 4))
                    pe_ns[0] += 2 * (qsz / 2.4 + 2.5)

            def finz(qi):
                qo, qsz = QN[qi]
                for hd in range(2):
                    lo = hd * 64
                    nc.vector.tensor_scalar_add(
                        zsl(b, hp, lo, 64, qo, qsz),
                        psz[(qi, hd)][0:64, :qsz],
                        bvc[lo:lo + 64, hp:hp + 1])
                    nc.vector.tensor_copy(
                        ddf[0:1, hd * T + qo:hd * T + qo + qsz],
                        psz[(qi, hd)][64:65, :qsz])

            # ---- emission sequence (ACT-paced: one fill per kt) ----
            sc(0, 0)
            sc(0, 1)
            fills()
            av(0, [0])
            sc(0, 2)
            fills()
            av(0, [1])
            sc(0, 3)
            fills()
            av(0, [2])
            sc(0, 4)
            fills()
            av(0, [3])
            av(0, [4])
            finz(0)
            sc(1, 0)
            fills()
            av(1, range(5))
            finz(1)
            # normalize: broadcast 1/denom via DRAM roundtrip, then scale z
            nc.vector.reciprocal_approx_fast(ddf[:], ddf[:])
            nc.vector.tensor_copy(ddb[:], ddf[:])
            dd = dram_p.tile([1, 2 * T], BF16, tag=f"dd{b}_{hp}",
                             name=f"dd{b}_{hp}")
            rb = dn_p.tile([128, T], BF16, tag="rb", name=f"rb{b}_{hp}")
            nc.sync.dma_start(dd[0:1, :], ddb[0:1, :])
            for hd in range(2):
                lo = hd * 64
                nc.sync.dma_start(
                    rb[lo:lo + 64, :],
                    dd[0:1, hd * T:hd * T + T].partition_broadcast(64))
                nc.gpsimd.tensor_mul(zsl(b, hp, lo, 64, 0, T),
                                     zsl(b, hp, lo, 64, 0, T), rb[lo:lo + 64, :])
            fills()

        # ---- the m-loop ----
        # W_O prefetch: reuses wv's buffer slot, DMA starts once V-phase is
        # done. Kept OFF the scalar engine: its DMA_DIRECT2D triggers cost
        # ~605ns of ACT time each and head-block exp gulps behind the
        # wtv-WAR semaphore right as the m-loop starts (ACT is the m-loop
        # pacer at ~87% busy).
        wot = wt_p.tile([128, MT * D], BF16, tag="wt", name="wot")
        for hp in range(MT):
            eng = nc.sync if hp % 2 == 0 else nc.gpsimd
            eng.dma_start(wot[:, hp * D:(hp + 1) * D],
                          wo.ap()[hp * 128:(hp + 1) * 128, :])
        for m in range(1, MT):
            make_a_units(m)
            emit_bhp(0, m - 1, fills=make_filler(8))
            emit_bhp(1, m - 1, fills=make_filler(8))
            fill(len(a_queue), force=True)  # drain leftovers to stay ahead
        # tail: last head pair; phase C interleaves below
        # ================= Phase C =================
        pa.close()  # frees psA (2 banks) before psO opens

        sbO_p = est.enter_context(tc.tile_pool(name="sbO", bufs=3))
        co = ExitStack()
        psO_p = co.enter_context(tc.tile_pool(name="psO", bufs=2, space="PSUM"))

        c_open = []  # dicts {b, mi, tiles, hp}
        c_queue = [(b, mi) for b in range(B) for mi in range(5)]

        def c_start(pool=None):
            pool = pool or psO_p
            b, mi = c_queue.pop(0)
            tiles = [pool.tile([128, 512], F32, tag="psO",
                               name=f"psO{b}_{mi}_{ni}") for ni in range(2)]
            c_open.append({"b": b, "mi": mi, "tiles": tiles, "hp": 0})

        def c_advance(nhp):
            u = c_open[-1]
            mo, msz = MO[u["mi"]]
            end = min(u["hp"] + nhp, MT - 1)
            for hp in range(u["hp"], end):
                for ni in range(2):
                    nc.tensor.matmul(
                        u["tiles"][ni][:msz, :],
                        zsl(u["b"], hp, 0, 128, mo, msz),
                        wot[:, hp * D + ni * 512:hp * D + ni * 512 + 512],
                        start=(hp == 0), stop=False)
                pe_ns[0] += 432.0
            u["hp"] = end

        def c_finish(mid_emit=False):
            u = c_open.pop(0)
            b, mi = u["b"], u["mi"]
            mo, msz = MO[mi]
            while u["hp"] < MT - 1:  # complete any unfinished prefix
                hp = u["hp"]
                for ni in range(2):
                    nc.tensor.matmul(
                        u["tiles"][ni][:msz, :],
                        zsl(b, hp, 0, 128, mo, msz),
                        wot[:, hp * D + ni * 512:hp * D + ni * 512 + 512],
                        start=(hp == 0), stop=False)
                pe_ns[0] += 432.0
                u["hp"] = hp + 1
            hp = MT - 1
            for ni in range(2):
                nc.tensor.matmul(
                    u["tiles"][ni][:msz, :],
                    zsl(b, hp, 0, 128, mo, msz),
                    wot[:, hp * D + ni * 512:hp * D + ni * 512 + 512],
                    start=False, stop=True)
            pe_ns[0] += 432.0
            for ni in range(2):
                so = sbO_p.tile([128, 512], BF16, tag="sbO",
                                name=f"sbO{b}_{mi}_{ni}")
                nc.vector.tensor_add(so[:msz, :], u["tiles"][ni][:msz, :],
                                     boc[:msz, ni * 512:ni * 512 + 512])
                # while exps are still running (mid-emit), keep the ni1 out
                # trigger off the scalar engine (it costs ~605ns of ACT time)
                eng = (nc.sync if ni == 0
                       else (nc.gpsimd if mid_emit else nc.scalar))
                eng.dma_start(
                    out.ap()[b, mo:mo + msz, ni * 512:ni * 512 + 512],
                    so[:msz, :])

        def cfill_b0(n=1):
            # during emit_bhp(0,7): prefix ONE b0 unit, granularly (3 hp/call).
            # A second open unit would WAR on the same psO banks.
            if not c_open and c_queue and c_queue[0][0] == 0:
                c_start()
            if c_open and c_open[-1]["hp"] < MT - 1:
                c_advance(3)

        b0_used = [0]

        def cfill_b1(n=1):
            # during emit_bhp(1,7): z(b0) fully ready. Alternate finish/open,
            # but keep >=2 b0 units in reserve to cover the b1-hp7 norm tail.
            if c_open and c_open[0]["hp"] >= MT - 1:
                c_finish(mid_emit=True)
            elif c_open and c_open[-1]["hp"] < MT - 1:
                c_advance(4)
            elif (not c_open and c_queue and c_queue[0][0] == 0
                  and b0_used[0] < 3):
                b0_used[0] += 1
                c_start()

        emit_bhp(0, MT - 1, fills=cfill_b0)
        emit_bhp(1, MT - 1, fills=cfill_b1)
        while c_open:
            c_finish()
        co.close()
        bs.close()
        psO3_p = est.enter_context(tc.tile_pool(name="psO3", bufs=4, space="PSUM"))
        while c_queue or c_open:
            if c_queue:
                c_start(pool=psO3_p)
                c_advance(MT - 1)
            if len(c_open) >= 2 or not c_queue:
                c_finish()


_GRAPH = None


def _get_graph():
    global _GRAPH
    if _GRAPH is None:
        _GRAPH = build_graph()
    return _GRAPH


def kernel(query_input, key_input, value_input, W_Q, W_K, W_V, W_O,
           b_Q, b_K, b_V, b_O, _trace=False, _trace_kwargs=None):
    import ml_dtypes
    from concourse.bass_utils import run_bass_kernel_spmd

    nc = _get_graph()
    f = np.ascontiguousarray
    bf = ml_dtypes.bfloat16

    def xT(x, sl):
        x = np.asarray(x[sl], np.float32)
        return f(x.reshape(B * T, D).T.astype(bf))

    def wT(w):
        w = np.asarray(w, np.float32)
        return f(w.transpose(1, 0, 2).reshape(D, HE).astype(bf))

    def bcol(bx):
        bx = np.asarray(bx, np.float32).reshape(HE)
        return f(bx.reshape(MT, 128).T)

    wq_m, wk_m, wv_m = wT(W_Q), wT(W_K), wT(W_V)
    wo_m = f(np.asarray(W_O, np.float32).reshape(HE, D).astype(bf))
    bq_m, bk_m, bv_m = bcol(b_Q), bcol(b_K), bcol(b_V)
    bo_m = f(np.asarray(b_O, np.float32).reshape(1, D))
    in_maps = []
    for c in range(NCORES):
        sl = slice(2 * c, 2 * c + 2)
        in_maps.append({
            "query_input": xT(query_input, sl),
            "key_input": xT(key_input, sl),
            "value_input": xT(value_input, sl),
            "W_Q": wq_m,
            "W_K": wk_m,
            "W_V": wv_m,
            "W_O": wo_m,
            "b_Q": bq_m,
            "b_K": bk_m,
            "b_V": bv_m,
            "b_O": bo_m,
        })
    res = run_bass_kernel_spmd(nc, in_maps, core_ids=list(range(NCORES)),
                               trace=_trace, **(_trace_kwargs or {}))
    outp = np.concatenate([np.asarray(res.results[c]["out"], np.float32)
                           for c in range(NCORES)], axis=0)
    if _trace:
        kernel._last_result = res
    return outp

